# revision 12
# baseline (speedup 1.0000x reference)
"""Trainium2 Bass kernel for causal self-attention (T=2048, C=1024, NH=16).

Strategy (tensor-parallel over heads, 2 heads/core on 8 cores):
  - Host pre-transposes x, w_attn-slice, w_proj so all device matmuls have
    their contraction dim on SBUF partitions (no fp32 DMA transposes needed).
  - Per core: qkv projection in [ch, t] layout; attention computed as
    att_T = k @ q.T tiles ([t_k, t_q]) so softmax's denominator comes for free
    from an appended ones-column on v (no partition-axis reduction).
  - Softmax without max-subtraction (inputs bounded, |att| < 8; mathematically
    identical), causal mask as additive -1e9 tiles fused pre-exp.
  - The reference's bug-faithful reshape (NH,T,HD)->swap(1,2)->(T,C) makes the
    output row-parallel over heads: each core produces 256 full output rows,
    so there is no all-reduce at all; host concatenates.
  - PE packing: the K=64 q@k matmuls for the two heads run concurrently in
    disjoint row-groups of the PE array (tile_position).
  - bf16 matmul operands (FWL weight loads, full PE rate); fp32 PSUM
    accumulation, masks, exp input, biases and softmax normalization.
"""
import math
import os

import numpy as np

import concourse.bass as bass
import concourse.bacc as bacc
import concourse.mybir as mybir
import concourse.tile as tile
from concourse import bass_utils
from concourse.masks import make_identity

T, C, NH, HD = 2048, 1024, 16, 64
P = 128
NCORES = 8
HPC = 2          # heads per core
F32 = mybir.dt.float32
MMDT = mybir.dt.bfloat16  # matmul input dtype
EXPF = mybir.ActivationFunctionType.Exp
NEG = -1.0e9


def _mm(ap):
    return ap


def _to_mm(a):
    import ml_dtypes
    return np.ascontiguousarray(np.asarray(a, dtype=np.float32).astype(ml_dtypes.bfloat16))


def build_nc():
    nc = bacc.Bacc(trn_type="TRN2", target_bir_lowering=False)

    xT_d = nc.dram_tensor("xT", [C, T], MMDT, kind="ExternalInput")
    wqkvT_d = nc.dram_tensor("wqkvT", [C, 3 * P], MMDT, kind="ExternalInput")
    bqkv_d = nc.dram_tensor("bqkv", [P, 3], F32, kind="ExternalInput")
    wprojT_d = nc.dram_tensor("wprojT", [C, C], MMDT, kind="ExternalInput")
    bproj_d = nc.dram_tensor("bproj", [1, C], F32, kind="ExternalInput")
    out_d = nc.dram_tensor("out", [2 * P, C], F32, kind="ExternalOutput")

    from contextlib import ExitStack

    with tile.TileContext(nc) as tc, ExitStack() as stack:
        consts = stack.enter_context(tc.tile_pool(name="consts", bufs=1))
        wpool = stack.enter_context(tc.tile_pool(name="wpool", bufs=1))
        main = stack.enter_context(tc.tile_pool(name="main", bufs=1))
        ps_misc = stack.enter_context(tc.tile_pool(name="ps_misc", bufs=2, space="PSUM"))
        ps_att = stack.enter_context(tc.tile_pool(name="ps_att", bufs=2, space="PSUM"))
        ps_y = stack.enter_context(tc.tile_pool(name="ps_y", bufs=2, space="PSUM"))

        # ---- constants ----
        with nc.named_scope("setup"):
            identity = consts.tile([P, P], F32)
            make_identity(nc, identity)
            # maskP[jp][p, 1024]: halves are mask_j for j=2jp, 2jp+1;
            # mask_j[p, f] = 0 if 128j + p <= f else -1e9
            maskPs = []
            for jp in range(2):
                maskP = consts.tile([P, 1024], F32, name=f"maskP{jp}")
                nc.gpsimd.memset(maskP, 0.0)
                for h in range(2):
                    j = 2 * jp + h
                    nc.gpsimd.affine_select(
                        out=maskP[:, 512 * h:512 * (h + 1)],
                        in_=maskP[:, 512 * h:512 * (h + 1)],
                        compare_op=mybir.AluOpType.is_ge,
                        fill=NEG,
                        base=-128 * j,
                        pattern=[[1, 512]],
                        channel_multiplier=-1,
                    )
                maskPs.append(maskP)
            bqkv_s = consts.tile([P, 3], F32)
            nc.sync.dma_start(out=bqkv_s, in_=bqkv_d.ap())
            bproj_bc = consts.tile([P, C], F32)
            bp = bproj_d.ap()
            bp_bcast = bass.AP(tensor=bp.tensor, offset=bp.offset,
                               ap=[[0, P]] + list(bp.ap[1:]))
            nc.sync.dma_start(out=bproj_bc, in_=bp_bcast)

        # ---- weights ----
        wqkvT_s = wpool.tile([P, 8, 3 * P], MMDT)
        nc.sync.dma_start(
            out=wqkvT_s, in_=wqkvT_d.ap().rearrange("(fo p) n -> p fo n", p=P)
        )
        wprojT_s = wpool.tile([P, 8, C], MMDT)

        q_t = main.tile([P, T], MMDT)
        k_t = main.tile([P, T], MMDT)
        v_augA = main.tile([P, 16, HD + 1], MMDT, name="v_augA")
        v_augB = main.tile([P, 16, HD + 1], MMDT, name="v_augB")
        v_augs = (v_augA, v_augB)
        ones_sb = consts.tile([P, 16, 1], F32)
        nc.vector.memset(ones_sb, 1.0)
        nc.vector.tensor_copy(v_augA[:, :, HD:HD + 1], ones_sb)
        nc.vector.tensor_copy(v_augB[:, :, HD:HD + 1], ones_sb)
        Y = main.tile([P, 8, 2 * P], MMDT)  # [m_part, mo, r_local]
        Y5 = Y.rearrange("p mo (l d two) -> p mo l d two", l=2, d=HD)

        with tc.tile_pool(name="px", bufs=1) as px:
            xT_s = px.tile([P, 8, T], MMDT)
            xT_r = xT_d.ap().rearrange("(fo p) t -> p fo t", p=P)
            # x in two halves along t so qkv matmuls overlap the DMA
            nc.sync.dma_start(out=xT_s[:, :, 0:1024], in_=xT_r[:, :, 0:1024])
            nc.sync.dma_start(out=xT_s[:, :, 1024:2048], in_=xT_r[:, :, 1024:2048])
            nc.sync.dma_start(
                out=wprojT_s, in_=wprojT_d.ap().rearrange("(fo p) n -> p fo n", p=P)
            )
            v_t = px.tile([P, T], F32)

            # ---- qkv projection: [ch, t] = wT.T @ xT ----
            with nc.named_scope("qkv"):
                dsts = (q_t, k_t, v_t)
                for half in range(2):
                    order = [(0, 2 * half), (1, 2 * half), (0, 2 * half + 1),
                             (1, 2 * half + 1), (2, 2 * half), (2, 2 * half + 1)]
                    for g, nt in order:
                        ps = ps_misc.tile([P, 512], F32, name="ps_mm", tag="mm")
                        for f in range(8):
                            nc.tensor.matmul(
                                ps,
                                lhsT=_mm(wqkvT_s[:, f, P * g:P * (g + 1)]),
                                rhs=_mm(xT_s[:, f, 512 * nt:512 * (nt + 1)]),
                                start=(f == 0),
                                stop=(f == 7),
                            )
                        nc.vector.tensor_scalar_add(
                            dsts[g][:, 512 * nt:512 * (nt + 1)], ps,
                            bqkv_s[:, g:g + 1],
                        )

            # ---- v_t -> v_nat (+ones col) via PE transpose, both heads at once ----
            with nc.named_scope("vT"):
                for b in range(16):
                    tp = ps_misc.tile([P, 512], F32, name="ps_tr", tag="mm")
                    nc.tensor.transpose(
                        tp[:, 0:P], v_t[:, P * b:P * (b + 1)], identity
                    )
                    nc.vector.tensor_copy(v_augA[:, b, 0:HD], tp[:, 0:HD])
                    nc.vector.tensor_copy(v_augB[:, b, 0:HD], tp[:, HD:2 * HD])

        # ---- attention ----
        with (
            tc.tile_pool(name="expp", bufs=6) as expp,
            tc.tile_pool(name="ytp", bufs=2) as ytp,
            tc.tile_pool(name="smallp", bufs=4) as smallp,
            tc.tile_pool(name="outp", bufs=2) as outp,
        ):
            with nc.named_scope("attn"):
                for bq in range(4):
                    nbk = 4 * bq + 4
                    y_ps = [
                        ps_y.tile([HD + 1, 512], F32, name=f"y_ps{l}", tag="y")
                        for l in range(HPC)
                    ]
                    for pair in range(nbk // 2):
                        att_ps = [
                            ps_att.tile([P, 1024], F32, name=f"att_ps{l}", tag="att")
                            for l in range(HPC)
                        ]
                        for h in range(2):
                            bk = 2 * pair + h
                            for l in range(HPC):
                                nc.tensor.matmul(
                                    att_ps[l][:, 512 * h:512 * (h + 1)],
                                    lhsT=_mm(k_t[HD * l:HD * (l + 1), P * bk:P * (bk + 1)]),
                                    rhs=_mm(q_t[HD * l:HD * (l + 1), 512 * bq:512 * (bq + 1)]),
                                    start=True,
                                    stop=True,
                                    tile_position=(HD * l, 0),
                                )
                        diag = 2 * pair >= 4 * bq
                        es_l = []
                        for l in range(HPC):
                            if diag:
                                nc.vector.tensor_add(
                                    att_ps[l], att_ps[l], maskPs[pair - 2 * bq]
                                )
                            es = expp.tile([P, 1024], MMDT, name="es", tag="es")
                            nc.scalar.activation(es, att_ps[l], EXPF)
                            es_l.append(es)
                        for l in range(HPC):
                            for h in range(2):
                                bk = 2 * pair + h
                                nc.tensor.matmul(
                                    y_ps[l],
                                    lhsT=_mm(v_augs[l][:, bk, :]),
                                    rhs=_mm(es_l[l][:, 512 * h:512 * (h + 1)]),
                                    start=(bk == 0),
                                    stop=(bk == nbk - 1),
                                )
                    # y_T -> y_nat, normalize, scatter into Y
                    for l in range(HPC):
                        yts = ytp.tile([HD + 1, 512], F32, name="yts", tag="yts")
                        nc.vector.tensor_copy(yts, y_ps[l])
                        for sub in range(4):
                            typ = ps_misc.tile([P, 512], F32, name="ps_ty", tag="mm")
                            nc.tensor.transpose(
                                typ[:, 0:HD + 1],
                                yts[:, P * sub:P * (sub + 1)],
                                identity[0:HD + 1, 0:HD + 1],
                            )
                            rc = smallp.tile([P, 1], F32, name="rc", tag="rc")
                            nc.vector.reciprocal(rc, typ[:, HD:HD + 1])
                            tb = 4 * bq + sub
                            phalf, mo = tb // 8, tb % 8
                            nc.vector.tensor_scalar_mul(
                                Y5[:, mo, l, :, phalf], typ[:, 0:HD], rc
                            )

            # ---- output projection (row-parallel; no collective) ----
            with nc.named_scope("proj"):
                for mt in range(2):
                    for nt in range(2):
                        ps = ps_misc.tile([P, 512], F32, name="ps_pr", tag="mm")
                        for mo in range(8):
                            nc.tensor.matmul(
                                ps,
                                lhsT=_mm(Y[:, mo, P * mt:P * (mt + 1)]),
                                rhs=_mm(wprojT_s[:, mo, 512 * nt:512 * (nt + 1)]),
                                start=(mo == 0),
                                stop=(mo == 7),
                            )
                        os_ = outp.tile([P, 512], F32, name="os", tag="os")
                        nc.vector.tensor_add(os_, ps, bproj_bc[:, 512 * nt:512 * (nt + 1)])
                        nc.sync.dma_start(
                            out=out_d.ap()[P * mt:P * (mt + 1), 512 * nt:512 * (nt + 1)],
                            in_=os_,
                        )

    nc.compile()
    return nc


_nc_cache = None


def kernel(**inputs):
    global _nc_cache
    x = np.ascontiguousarray(np.asarray(inputs["x"], dtype=np.float32))
    w_attn = np.asarray(inputs["w_attn"], dtype=np.float32)
    b_attn = np.asarray(inputs["b_attn"], dtype=np.float32)
    w_proj = np.asarray(inputs["w_proj"], dtype=np.float32)
    b_proj = np.asarray(inputs["b_proj"], dtype=np.float32)

    scale = 1.0 / math.sqrt(HD)
    xT = _to_mm(x.T)
    wprojT = _to_mm(w_proj.T)
    bproj_r = np.ascontiguousarray(b_proj[None, :])

    in_maps = []
    for c in range(NCORES):
        ch0 = P * c
        wq = w_attn[ch0:ch0 + P, :] * scale
        wk = w_attn[C + ch0:C + ch0 + P, :]
        wv = w_attn[2 * C + ch0:2 * C + ch0 + P, :]
        wqkvT = _to_mm(np.concatenate([wq, wk, wv], axis=0).T)
        bqkv = np.ascontiguousarray(
            np.stack(
                [
                    b_attn[ch0:ch0 + P] * scale,
                    b_attn[C + ch0:C + ch0 + P],
                    b_attn[2 * C + ch0:2 * C + ch0 + P],
                ],
                axis=1,
            )
        )
        in_maps.append(
            {
                "xT": xT,
                "wqkvT": wqkvT,
                "bqkv": bqkv,
                "wprojT": wprojT,
                "bproj": bproj_r,
            }
        )

    if _nc_cache is None:
        _nc_cache = build_nc()
    nc = _nc_cache

    trace = os.environ.get("BASS_KERNEL_TRACE", "0") == "1"
    res = bass_utils.run_bass_kernel_spmd(
        nc, in_maps, core_ids=list(range(NCORES)), trace=trace
    )
    if trace:
        print(f"HW exec time: {res.exec_time_ns} ns")
        if res.per_core_scope_times:
            for scope, times in sorted(res.per_core_scope_times.items()):
                print(f"  scope {scope}: {times}")
        if res.instructions_and_trace:
            print(f"  trace: {res.instructions_and_trace[1]}")

    out = np.concatenate([r["out"] for r in res.results], axis=0)
    return np.ascontiguousarray(out.astype(np.float32))


if __name__ == "__main__":
    nc = build_nc()
    print("build OK:", len(nc.m.functions[0].basicblocks[0].instructions) if hasattr(nc.m.functions[0], 'basicblocks') else "n/a")


# revision 14
# speedup vs baseline: 1.4113x; 1.4113x over previous
"""Trainium2 Bass kernel for causal self-attention (T=2048, C=1024, NH=16).

Strategy (tensor-parallel over heads, 2 heads/core on 8 cores):
  - Host pre-transposes x, w_attn-slice, w_proj so all device matmuls have
    their contraction dim on SBUF partitions (no fp32 DMA transposes needed).
  - Per core: qkv projection in [ch, t] layout; attention computed as
    att_T = k @ q.T tiles ([t_k, t_q]) so softmax's denominator comes for free
    from an appended ones-column on v (no partition-axis reduction).
  - Softmax without max-subtraction (inputs bounded, |att| < 8; mathematically
    identical), causal mask as additive -1e9 tiles fused pre-exp.
  - The reference's bug-faithful reshape (NH,T,HD)->swap(1,2)->(T,C) makes the
    output row-parallel over heads: each core produces 256 full output rows,
    so there is no all-reduce at all; host concatenates.
  - PE packing: the K=64 q@k matmuls for the two heads run concurrently in
    disjoint row-groups of the PE array (tile_position).
  - bf16 matmul operands (FWL weight loads, full PE rate); fp32 PSUM
    accumulation, masks, exp input, biases and softmax normalization.
"""
import math
import os

import numpy as np

import concourse.bass as bass
import concourse.bacc as bacc
import concourse.mybir as mybir
import concourse.tile as tile
from concourse import bass_utils
from concourse.masks import make_identity

T, C, NH, HD = 2048, 1024, 16, 64
P = 128
NCORES = 8
HPC = 2          # heads per core
F32 = mybir.dt.float32
MMDT = mybir.dt.bfloat16  # matmul input dtype
EXPF = mybir.ActivationFunctionType.Exp
NEG = -1.0e9


def _mm(ap):
    return ap


def _to_mm(a):
    import ml_dtypes
    return np.ascontiguousarray(np.asarray(a, dtype=np.float32).astype(ml_dtypes.bfloat16))


def build_nc():
    nc = bacc.Bacc(trn_type="TRN2", target_bir_lowering=False)

    xT_d = nc.dram_tensor("xT", [C, T], MMDT, kind="ExternalInput")
    wqkvT_d = nc.dram_tensor("wqkvT", [C, 3 * P], MMDT, kind="ExternalInput")
    bqkv_d = nc.dram_tensor("bqkv", [P, 3], F32, kind="ExternalInput")
    wprojT_d = nc.dram_tensor("wprojT", [C, C], MMDT, kind="ExternalInput")
    bproj_d = nc.dram_tensor("bproj", [1, C], F32, kind="ExternalInput")
    out_d = nc.dram_tensor("out", [2 * P, C], F32, kind="ExternalOutput")

    from contextlib import ExitStack

    with tile.TileContext(nc) as tc, ExitStack() as stack:
        consts = stack.enter_context(tc.tile_pool(name="consts", bufs=1))
        wpool = stack.enter_context(tc.tile_pool(name="wpool", bufs=1))
        main = stack.enter_context(tc.tile_pool(name="main", bufs=1))
        ps_misc = stack.enter_context(tc.tile_pool(name="ps_misc", bufs=2, space="PSUM"))
        ps_att = stack.enter_context(tc.tile_pool(name="ps_att", bufs=2, space="PSUM"))
        ps_y = stack.enter_context(tc.tile_pool(name="ps_y", bufs=2, space="PSUM"))

        # ---- constants ----
        with nc.named_scope("setup"):
            identity = consts.tile([P, P], F32)
            make_identity(nc, identity)
            # multiplicative causal masks (1.0 valid / 0.0 masked), applied to
            # exp output; halves are mask_j for j=2jp, 2jp+1 where
            # mask_j[p, f] = 1.0 if 128j + p <= f else 0.0
            maskPs = []
            for jp in range(2):
                maskP = consts.tile([P, 1024], MMDT, name=f"maskP{jp}")
                nc.gpsimd.memset(maskP, 1.0)
                for h in range(2):
                    j = 2 * jp + h
                    nc.gpsimd.affine_select(
                        out=maskP[:, 512 * h:512 * (h + 1)],
                        in_=maskP[:, 512 * h:512 * (h + 1)],
                        compare_op=mybir.AluOpType.is_ge,
                        fill=0.0,
                        base=-128 * j,
                        pattern=[[1, 512]],
                        channel_multiplier=-1,
                    )
                maskPs.append(maskP)
            bqkv_s = consts.tile([P, 3], F32)
            nc.sync.dma_start(out=bqkv_s, in_=bqkv_d.ap())
            bproj_bc = consts.tile([P, C], F32)
            bp = bproj_d.ap()
            bp_bcast = bass.AP(tensor=bp.tensor, offset=bp.offset,
                               ap=[[0, P]] + list(bp.ap[1:]))
            nc.sync.dma_start(out=bproj_bc, in_=bp_bcast)

        # ---- weights ----
        wqkvT_s = wpool.tile([P, 8, 3 * P], MMDT)
        nc.sync.dma_start(
            out=wqkvT_s, in_=wqkvT_d.ap().rearrange("(fo p) n -> p fo n", p=P)
        )
        wprojT_s = wpool.tile([P, 8, C], MMDT)

        # per-head q/k tiles, zero-padded to K=128 so q@k.T uses the full PE
        # array (head A data in partitions 0:64, head B in 64:128)
        qA = main.tile([P, T], MMDT, name="qA")
        qB = main.tile([P, T], MMDT, name="qB")
        kA = main.tile([P, T], MMDT, name="kA")
        kB = main.tile([P, T], MMDT, name="kB")
        nc.vector.memset(qA[HD:P, :], 0.0)
        nc.vector.memset(qB[0:HD, :], 0.0)
        nc.vector.memset(kA[HD:P, :], 0.0)
        nc.vector.memset(kB[0:HD, :], 0.0)
        q_tiles, k_tiles = (qA, qB), (kA, kB)
        v_t = main.tile([P, T], F32)
        # v_aug padded to 128 columns (cols 0:64 v, col 64 ones, rest zero) so
        # att@v is a full-array matmul
        v_augA = main.tile([P, 16, P], MMDT, name="v_augA")
        v_augB = main.tile([P, 16, P], MMDT, name="v_augB")
        v_augs = (v_augA, v_augB)
        nc.vector.memset(v_augA, 0.0)
        nc.vector.memset(v_augB, 0.0)
        ones_sb = consts.tile([P, 16, 1], F32)
        nc.vector.memset(ones_sb, 1.0)
        nc.vector.tensor_copy(v_augA[:, :, HD:HD + 1], ones_sb)
        nc.vector.tensor_copy(v_augB[:, :, HD:HD + 1], ones_sb)
        Y = main.tile([P, 8, 2 * P], MMDT)  # [m_part, mo, r_local]
        Y5 = Y.rearrange("p mo (l d two) -> p mo l d two", l=2, d=HD)

        with tc.tile_pool(name="px", bufs=1) as px:
            xT_s = px.tile([P, 8, T], MMDT)
            xT_r = xT_d.ap().rearrange("(fo p) t -> p fo t", p=P)
            # x in four quarters along t so qkv matmuls overlap the DMA
            for qtr in range(4):
                nc.sync.dma_start(out=xT_s[:, :, 512 * qtr:512 * (qtr + 1)],
                                  in_=xT_r[:, :, 512 * qtr:512 * (qtr + 1)])
            nc.sync.dma_start(
                out=wprojT_s, in_=wprojT_d.ap().rearrange("(fo p) n -> p fo n", p=P)
            )

            # ---- qkv projection: [ch, t] = wT.T @ xT ----
            with nc.named_scope("qkv"):
                for nt in range(4):
                    for g in range(3):
                        ps = ps_misc.tile([P, 512], F32, name="ps_mm", tag="mm")
                        for f in range(8):
                            nc.tensor.matmul(
                                ps,
                                lhsT=wqkvT_s[:, f, P * g:P * (g + 1)],
                                rhs=xT_s[:, f, 512 * nt:512 * (nt + 1)],
                                start=(f == 0),
                                stop=(f == 7),
                            )
                        ts = slice(512 * nt, 512 * (nt + 1))
                        if g < 2:
                            dA, dB = (qA, qB) if g == 0 else (kA, kB)
                            nc.vector.tensor_scalar_add(
                                dA[0:HD, ts], ps[0:HD, :], bqkv_s[0:HD, g:g + 1])
                            nc.vector.tensor_scalar_add(
                                dB[HD:P, ts], ps[HD:P, :], bqkv_s[HD:P, g:g + 1])
                        else:
                            nc.vector.tensor_scalar_add(
                                v_t[:, ts], ps, bqkv_s[:, 2:3])

        # ---- attention ----
        with (
            tc.tile_pool(name="expp", bufs=6) as expp,
            tc.tile_pool(name="ytp", bufs=2) as ytp,
            tc.tile_pool(name="smallp", bufs=4) as smallp,
            tc.tile_pool(name="outp", bufs=2) as outp,
        ):
            with nc.named_scope("attn"):
                for bq in range(4):
                    # v_t -> v_aug transposes for the 4 new t_k blocks this
                    # round needs (interleaved to avoid a long PE-idle phase)
                    for b in range(4 * bq, 4 * bq + 4):
                        tp = ps_misc.tile([P, 512], F32, name="ps_tr", tag="mm")
                        nc.tensor.transpose(
                            tp[:, 0:P], v_t[:, P * b:P * (b + 1)], identity
                        )
                        nc.vector.tensor_copy(v_augA[:, b, 0:HD], tp[:, 0:HD])
                        nc.vector.tensor_copy(v_augB[:, b, 0:HD], tp[:, HD:2 * HD])
                    nbk = 4 * bq + 4
                    y_ps = [
                        ps_y.tile([P, 512], F32, name=f"y_ps{l}", tag="y")
                        for l in range(HPC)
                    ]
                    for pair in range(nbk // 2):
                        att_ps = [
                            ps_att.tile([P, 1024], F32, name=f"att_ps{l}", tag="att")
                            for l in range(HPC)
                        ]
                        for h in range(2):
                            bk = 2 * pair + h
                            for l in range(HPC):
                                nc.tensor.matmul(
                                    att_ps[l][:, 512 * h:512 * (h + 1)],
                                    lhsT=k_tiles[l][:, P * bk:P * (bk + 1)],
                                    rhs=q_tiles[l][:, 512 * bq:512 * (bq + 1)],
                                    start=True,
                                    stop=True,
                                )
                        diag = 2 * pair >= 4 * bq
                        es_l = []
                        for l in range(HPC):
                            es = expp.tile([P, 1024], MMDT, name="es", tag="es")
                            nc.scalar.activation(es, att_ps[l], EXPF)
                            if diag:
                                nc.vector.tensor_mul(es, es, maskPs[pair - 2 * bq])
                            es_l.append(es)
                        for l in range(HPC):
                            for h in range(2):
                                bk = 2 * pair + h
                                nc.tensor.matmul(
                                    y_ps[l],
                                    lhsT=v_augs[l][:, bk, :],
                                    rhs=es_l[l][:, 512 * h:512 * (h + 1)],
                                    start=(bk == 0),
                                    stop=(bk == nbk - 1),
                                )
                    # y_T -> y_nat, normalize, scatter into Y
                    for l in range(HPC):
                        yts = ytp.tile([HD + 1, 512], F32, name="yts", tag="yts")
                        nc.vector.tensor_copy(yts, y_ps[l][0:HD + 1, :])
                        for sub in range(4):
                            typ = ps_misc.tile([P, 512], F32, name="ps_ty", tag="mm")
                            nc.tensor.transpose(
                                typ[:, 0:HD + 1],
                                yts[:, P * sub:P * (sub + 1)],
                                identity[0:HD + 1, 0:HD + 1],
                            )
                            rc = smallp.tile([P, 1], F32, name="rc", tag="rc")
                            nc.vector.reciprocal(rc, typ[:, HD:HD + 1])
                            tb = 4 * bq + sub
                            phalf, mo = tb // 8, tb % 8
                            nc.vector.tensor_scalar_mul(
                                Y5[:, mo, l, :, phalf], typ[:, 0:HD], rc
                            )

            # ---- output projection (row-parallel; no collective) ----
            with nc.named_scope("proj"):
                for mt in range(2):
                    for nt in range(2):
                        ps = ps_misc.tile([P, 512], F32, name="ps_pr", tag="mm")
                        for mo in range(8):
                            nc.tensor.matmul(
                                ps,
                                lhsT=Y[:, mo, P * mt:P * (mt + 1)],
                                rhs=wprojT_s[:, mo, 512 * nt:512 * (nt + 1)],
                                start=(mo == 0),
                                stop=(mo == 7),
                            )
                        os_ = outp.tile([P, 512], F32, name="os", tag="os")
                        nc.vector.tensor_add(os_, ps, bproj_bc[:, 512 * nt:512 * (nt + 1)])
                        nc.sync.dma_start(
                            out=out_d.ap()[P * mt:P * (mt + 1), 512 * nt:512 * (nt + 1)],
                            in_=os_,
                        )

    nc.compile()
    return nc


_nc_cache = None


def kernel(**inputs):
    global _nc_cache
    x = np.ascontiguousarray(np.asarray(inputs["x"], dtype=np.float32))
    w_attn = np.asarray(inputs["w_attn"], dtype=np.float32)
    b_attn = np.asarray(inputs["b_attn"], dtype=np.float32)
    w_proj = np.asarray(inputs["w_proj"], dtype=np.float32)
    b_proj = np.asarray(inputs["b_proj"], dtype=np.float32)

    scale = 1.0 / math.sqrt(HD)
    xT = _to_mm(x.T)
    wprojT = _to_mm(w_proj.T)
    bproj_r = np.ascontiguousarray(b_proj[None, :])

    in_maps = []
    for c in range(NCORES):
        ch0 = P * c
        wq = w_attn[ch0:ch0 + P, :] * scale
        wk = w_attn[C + ch0:C + ch0 + P, :]
        wv = w_attn[2 * C + ch0:2 * C + ch0 + P, :]
        wqkvT = _to_mm(np.concatenate([wq, wk, wv], axis=0).T)
        bqkv = np.ascontiguousarray(
            np.stack(
                [
                    b_attn[ch0:ch0 + P] * scale,
                    b_attn[C + ch0:C + ch0 + P],
                    b_attn[2 * C + ch0:2 * C + ch0 + P],
                ],
                axis=1,
            )
        )
        in_maps.append(
            {
                "xT": xT,
                "wqkvT": wqkvT,
                "bqkv": bqkv,
                "wprojT": wprojT,
                "bproj": bproj_r,
            }
        )

    if _nc_cache is None:
        _nc_cache = build_nc()
    nc = _nc_cache

    trace = os.environ.get("BASS_KERNEL_TRACE", "0") == "1"
    res = bass_utils.run_bass_kernel_spmd(
        nc, in_maps, core_ids=list(range(NCORES)), trace=trace
    )
    if trace:
        print(f"HW exec time: {res.exec_time_ns} ns")
        if res.per_core_scope_times:
            for scope, times in sorted(res.per_core_scope_times.items()):
                print(f"  scope {scope}: {times}")
        if res.instructions_and_trace:
            print(f"  trace: {res.instructions_and_trace[1]}")

    out = np.concatenate([r["out"] for r in res.results], axis=0)
    return np.ascontiguousarray(out.astype(np.float32))


if __name__ == "__main__":
    nc = build_nc()
    print("build OK:", len(nc.m.functions[0].basicblocks[0].instructions) if hasattr(nc.m.functions[0], 'basicblocks') else "n/a")
